# revision 1
# baseline (speedup 1.0000x reference)
"""Trainium2 Bass kernel for nn_HHGNN_17179869473.

Computation (per batch b):
    y_t   = embeds @ W[t] + b[t]            (per-type linear, routed by node type)
    x     = where(mask, y_type[n], embeds)
    x     = LayerNorm(x) * gamma + beta
    lat   = adj^T @ x                        [M, H]
    ret   = adj @ lat                        [N, H]

Strategy: pure data parallel — batch b -> core b (B == 8 == n_cores).
Host-side prep per batch:
  * stable-sort nodes by type; permute adj rows / embeds rows accordingly.
    Every 128-node block then contains at most a couple of distinct types, so
    the per-type projection needs only SLOTS (=max types per block) matmuls
    per block instead of 13, with a per-partition select merge.
  * pre-transpose adj (fp16) so the second aggregation pass streams m-major
    without any on-device transposition of the 32MB matrix.
  * fp16 for adj / x / lat matmul operands (measured end-to-end max rel err
    ~1.4e-4 vs fp32 reference; fp32 accumulation in PSUM throughout).
Device phases (per core):
  P1: proj matmuls (stationary = embT block, moving = per-block W pair) +
      bias/select/mask merge on DVE + per-block LayerNorm -> x fp16.
  P2: latT[h, m] accumulation: stationary = x block, moving = adj row-block.
  P3: lat blocks transposed on PE, then retT[h, n] accumulation:
      stationary = lat block, moving = adjT row-block.  retT -> DRAM.
Host post: out[b] = unpermute(retT.T).
"""

import os
import sys

for _p in ("/opt/trn_rl_repo", "/root/.axon_site/_ro/trn_rl_repo"):
    if os.path.isdir(_p) and _p not in sys.path:
        sys.path.insert(0, _p)

import numpy as np

import concourse.bacc as bacc
import concourse.tile as tile
from concourse import mybir
from concourse.bass_interp import get_hw_module
from concourse.bass_utils import run_bass_kernel_spmd

B, N, M, H, T = 8, 4096, 2048, 128, 13
NBLK = N // 128          # 32 node blocks
MBLK = M // 128          # 16 m blocks
MCH = M // 512           # 4 moving chunks over m
NCH = N // 512           # 8 moving chunks over n
LN_EPS = 1e-5
F32 = mybir.dt.float32
F16 = mybir.dt.float16
ADD = mybir.AluOpType.add
SUB = mybir.AluOpType.subtract
MUL = mybir.AluOpType.mult
AF = mybir.ActivationFunctionType

_PROGRAM_CACHE = {}


def _build_program(S):
    """Build the (SPMD-uniform) Bass program for SLOTS == S."""
    nc = bacc.Bacc(
        "TRN2",
        target_bir_lowering=False,
        debug=False,
        enable_asserts=False,
        num_devices=B,
    )

    adj_d = nc.dram_tensor("adj", [N, M], F16, kind="ExternalInput").ap()
    adjT_d = nc.dram_tensor("adjT", [M, N], F16, kind="ExternalInput").ap()
    embT_d = nc.dram_tensor("embT", [H, N], F16, kind="ExternalInput").ap()
    embP_d = nc.dram_tensor("embP", [128, N], F16, kind="ExternalInput").ap()
    brP_d = nc.dram_tensor("brP", [128, N], F16, kind="ExternalInput").ap()
    Wblk_d = nc.dram_tensor("Wblk", [128, NBLK * S * 128], F16, kind="ExternalInput").ap()
    selP_d = nc.dram_tensor("selP", [128, NBLK * (S + 1)], F32, kind="ExternalInput").ap()
    gb_d = nc.dram_tensor("gb", [128, H], F32, kind="ExternalInput").ap()
    bb_d = nc.dram_tensor("bb", [128, H], F32, kind="ExternalInput").ap()
    id_d = nc.dram_tensor("ident", [128, 128], F16, kind="ExternalInput").ap()
    ret_d = nc.dram_tensor("retT", [H, N], F32, kind="ExternalOutput").ap()

    with tile.TileContext(nc, trace_sim=False) as tc:
        with (
            tc.tile_pool(name="const", bufs=1) as constp,
            tc.tile_pool(name="xpool", bufs=1) as xpool,
            tc.tile_pool(name="scr", bufs=3) as scr,
            tc.tile_pool(name="adjp", bufs=5) as adjp,
            tc.tile_pool(name="adjtp", bufs=6) as adjtp,
            tc.tile_pool(name="outp", bufs=3) as outp,
        ):
            # ---- replicated / per-core constants (SWDGE path keeps the HWDGE
            # FIFO free for the adj streams) ----
            embT_sb = constp.tile([128, N], F16)
            nc.gpsimd.dma_start(embT_sb[:], embT_d)
            Wblk_sb = constp.tile([128, NBLK * S * 128], F16)
            nc.gpsimd.dma_start(Wblk_sb[:], Wblk_d)
            embP_sb = constp.tile([128, N], F16)
            nc.gpsimd.dma_start(embP_sb[:], embP_d)
            brP_sb = constp.tile([128, N], F16)
            nc.gpsimd.dma_start(brP_sb[:], brP_d)
            selP_sb = constp.tile([128, NBLK * (S + 1)], F32)
            nc.gpsimd.dma_start(selP_sb[:], selP_d)
            gb_sb = constp.tile([128, H], F32)
            nc.gpsimd.dma_start(gb_sb[:], gb_d)
            bb_sb = constp.tile([128, H], F32)
            nc.gpsimd.dma_start(bb_sb[:], bb_d)
            id_sb = constp.tile([128, 128], F16)
            nc.gpsimd.dma_start(id_sb[:], id_d)
            eps_sb = constp.tile([128, 1], F32)
            nc.vector.memset(eps_sb[:], LN_EPS)

            x_sb = xpool.tile([128, N], F16)      # LN output, block-packed

            # ---------------- P1: projection + route + LayerNorm -------------
            ps1_cm = tc.tile_pool(name="ps1", bufs=2, space="PSUM")
            ps2_cm = tc.tile_pool(name="ps2", bufs=1, space="PSUM")
            ps1 = ps1_cm.__enter__()
            ps2 = ps2_cm.__enter__()
            for k in range(NBLK):
                pp = ps1.tile([128, S * 128], F32)
                base = k * S * 128
                off = 0
                while off < S * 128:
                    w = min(512, S * 128 - off)
                    nc.tensor.matmul(
                        pp[:, off:off + w],
                        embT_sb[:, k * 128:(k + 1) * 128],
                        Wblk_sb[:, base + off:base + off + w],
                        start=True,
                        stop=True,
                    )
                    off += w

                def sel(c):
                    j = k * (S + 1) + c
                    return selP_sb[:, j:j + 1]

                # xm = sum_s proj_s * sel_s  +  emb * notmask + b_routed
                prev = scr.tile([128, 128], F32, tag="merge")
                nc.vector.scalar_tensor_tensor(
                    prev[:], embP_sb[:, k * 128:(k + 1) * 128], sel(S),
                    brP_sb[:, k * 128:(k + 1) * 128], op0=MUL, op1=ADD)
                for s in range(S):
                    cur = scr.tile([128, 128], F32, tag="merge")
                    nc.vector.scalar_tensor_tensor(
                        cur[:], pp[:, s * 128:(s + 1) * 128], sel(s), prev[:],
                        op0=MUL, op1=ADD)
                    prev = cur
                xm = prev

                # per-node LayerNorm over the free (h) axis
                s_t = scr.tile([128, 1], F32, tag="s")
                nc.vector.tensor_reduce(s_t[:], xm[:], axis=mybir.AxisListType.X, op=ADD)
                sq_t = scr.tile([128, 128], F32, tag="sq")
                ssq_t = scr.tile([128, 1], F32, tag="ssq")
                nc.scalar.activation(sq_t[:], xm[:], AF.Square, accum_out=ssq_t[:])
                mu_t = scr.tile([128, 1], F32, tag="mu")
                nc.vector.tensor_scalar_mul(mu_t[:], s_t[:], 1.0 / H)
                e2_t = scr.tile([128, 1], F32, tag="e2")
                nc.vector.tensor_scalar_mul(e2_t[:], ssq_t[:], 1.0 / H)
                nv_t = scr.tile([128, 1], F32, tag="nv")
                nc.vector.tensor_scalar(nv_t[:], mu_t[:], mu_t[:], e2_t[:], op0=MUL, op1=SUB)
                std_t = scr.tile([128, 1], F32, tag="std")
                nc.scalar.activation(std_t[:], nv_t[:], AF.Sqrt, bias=eps_sb[:], scale=-1.0)
                r_t = scr.tile([128, 1], F32, tag="r")
                nc.vector.reciprocal(r_t[:], std_t[:])
                a_t = scr.tile([128, 128], F32, tag="a")
                nc.vector.scalar_tensor_tensor(a_t[:], xm[:], mu_t[:], gb_sb[:], op0=SUB, op1=MUL)
                nc.vector.scalar_tensor_tensor(
                    x_sb[:, k * 128:(k + 1) * 128], a_t[:], r_t[:], bb_sb[:], op0=MUL, op1=ADD)

            # ---------------- P2: latT[h, m] = sum_n x^T adj -----------------
            lat_ps = ps2.tile([128, M], F32)
            for kk in range(NBLK // 2):
                at = adjp.tile([128, 2 * M], F16)
                eng = nc.sync if kk % 2 == 0 else nc.scalar
                src = adj_d[kk * 256:(kk + 1) * 256, :].rearrange(
                    "(two p) m -> p two m", p=128)
                eng.dma_start(at[:].rearrange("p (two m) -> p two m", two=2), src)
                for t in range(2):
                    k = kk * 2 + t
                    for c in range(MCH):
                        nc.tensor.matmul(
                            lat_ps[:, c * 512:(c + 1) * 512],
                            x_sb[:, k * 128:(k + 1) * 128],
                            at[:, t * M + c * 512:t * M + (c + 1) * 512],
                            start=(k == 0),
                            stop=(k == NBLK - 1),
                        )
            latT_sb = xpool.tile([128, M], F16)
            nc.scalar.copy(latT_sb[:], lat_ps[:])
            ps2_cm.__exit__(None, None, None)
            ps1_cm.__exit__(None, None, None)

            # transpose lat to natural [m, h] blocks (PE transpose)
            lat_sb = xpool.tile([128, MBLK * 128], F16)
            with tc.tile_pool(name="pst", bufs=2, space="PSUM") as pst:
                for mb in range(MBLK):
                    pt = pst.tile([128, 128], F16)
                    nc.tensor.transpose(pt[:], latT_sb[:, mb * 128:(mb + 1) * 128], id_sb[:])
                    nc.vector.tensor_copy(lat_sb[:, mb * 128:(mb + 1) * 128], pt[:])

            # ---------------- P3: retT[h, n] = sum_m lat^T adjT --------------
            ps3_cm = tc.tile_pool(name="ps3", bufs=1, space="PSUM")
            ps3 = ps3_cm.__enter__()
            ret_ps = ps3.tile([128, N], F32)
            for mb in range(MBLK):
                att = adjtp.tile([128, N], F16)
                (nc.scalar if mb % 2 == 0 else nc.sync).dma_start(att[:], adjT_d[mb * 128:(mb + 1) * 128, :])
                for c in range(NCH):
                    nc.tensor.matmul(
                        ret_ps[:, c * 512:(c + 1) * 512],
                        lat_sb[:, mb * 128:(mb + 1) * 128],
                        att[:, c * 512:(c + 1) * 512],
                        start=(mb == 0),
                        stop=(mb == MBLK - 1),
                    )
            for c in range(NCH):
                rt = outp.tile([128, 512], F32, tag="rt")
                nc.vector.tensor_copy(rt[:], ret_ps[:, c * 512:(c + 1) * 512])
                nc.scalar.dma_start(ret_d[:, c * 512:(c + 1) * 512], rt[:])
            ps3_cm.__exit__(None, None, None)

    nc.compile()
    nc.m = get_hw_module(nc.m)
    return nc


def _prep_core(adj_b, emb_b, type_b, mask_b, W, b, S):
    """Host-side input marshalling for one batch."""
    perm = np.argsort(type_b, kind="stable")
    t_s = type_b[perm]
    m_s = mask_b[perm].astype(np.float32)
    adj_p = adj_b[perm].astype(np.float16)          # [N, M]
    adjT_p = np.ascontiguousarray(adj_p.T)          # [M, N]
    emb_p = emb_b[perm]                             # [N, H] fp32
    embT = np.ascontiguousarray(emb_p.T.astype(np.float16))  # [H, N]

    def pack(a):  # [N, H] -> [128, NBLK*H] block-major pack
        return np.ascontiguousarray(
            a.reshape(NBLK, 128, -1).transpose(1, 0, 2).reshape(128, -1))

    embP = pack(emb_p).astype(np.float16)
    brP = pack(b[t_s] * m_s[:, None]).astype(np.float16)

    Wblk = np.zeros((128, NBLK * S * 128), np.float16)
    selP = np.zeros((128, NBLK * (S + 1)), np.float32)
    for k in range(NBLK):
        blk_t = t_s[k * 128:(k + 1) * 128]
        blk_m = m_s[k * 128:(k + 1) * 128]
        uniq = np.unique(blk_t)
        assert len(uniq) <= S
        for s, tt in enumerate(uniq):
            Wblk[:, (k * S + s) * 128:(k * S + s + 1) * 128] = W[tt].astype(np.float16)
            selP[:, k * (S + 1) + s] = (blk_t == tt) * blk_m
        selP[:, k * (S + 1) + S] = 1.0 - blk_m

    return perm, {
        "adj": adj_p,
        "adjT": adjT_p,
        "embT": embT,
        "embP": embP,
        "brP": brP,
        "Wblk": Wblk,
        "selP": selP,
    }


def kernel(adj, embeds, node_type_index, node_mask, W, b, gamma, beta):
    adj = np.asarray(adj, np.float32)
    embeds = np.asarray(embeds, np.float32)
    node_type_index = np.asarray(node_type_index)
    node_mask = np.asarray(node_mask)
    W = np.asarray(W, np.float32)
    b = np.asarray(b, np.float32)
    gamma = np.asarray(gamma, np.float32)
    beta = np.asarray(beta, np.float32)

    # SLOTS = max distinct node types within any sorted 128-node block
    S = 1
    for bi in range(B):
        t_sorted = np.sort(node_type_index[bi])
        for k in range(NBLK):
            S = max(S, len(np.unique(t_sorted[k * 128:(k + 1) * 128])))

    common = {
        "gb": np.ascontiguousarray(np.broadcast_to(gamma, (128, H))).astype(np.float32),
        "bb": np.ascontiguousarray(np.broadcast_to(beta, (128, H))).astype(np.float32),
        "ident": np.eye(128, dtype=np.float16),
    }
    perms = []
    in_maps = []
    for bi in range(B):
        perm, m = _prep_core(
            adj[bi], embeds[bi], node_type_index[bi], node_mask[bi], W, b, S)
        perms.append(perm)
        m.update(common)
        in_maps.append(m)

    if S not in _PROGRAM_CACHE:
        _PROGRAM_CACHE[S] = _build_program(S)
    nc = _PROGRAM_CACHE[S]

    res = run_bass_kernel_spmd(nc, in_maps, core_ids=list(range(B)))
    kernel.last_results = res
    kernel.last_nc = nc

    out = np.empty((B, N, H), np.float32)
    for bi in range(B):
        ret_sorted = res.results[bi]["retT"].T      # [N, H] in sorted order
        out[bi][perms[bi]] = ret_sorted
    return out

